# revision 1
# baseline (speedup 1.0000x reference)
"""Fused conv1x1-attention-FFN kernel for 8 trn2 NeuronCores.

Reference computation (per batch b of 4, N = 64*64 = 4096 pixels, C = 256):
    q = Wq @ x_q + bq ; k = Wk @ x_kv + bk ; v = Wv @ x_kv + bv      [C, N]
    attn = softmax_over_keys(q^T k)                                   [N, N]
    av = v @ attn^T                                                   [C, N]
    out = W2 @ relu(W1 @ av + b1) + b2                                [C, N]

Sharding: 8 cores = 4 batches x 2 query-row halves. Each core computes the
full K/V for its batch (cheap: 0.5 GMAC vs 4.3 GMAC attention) and attends
its 2048 query rows against all 4096 keys — no collectives needed.

On-chip layout (all matmuls contract over the partition dim):
    scores are computed TRANSPOSED: S^T[m, n] = sum_c k[c,m] q[c,n] so that
    the attention-value product av[c,n] = sum_m v^T[m,c] E[m,n] needs no
    on-chip transpose; v is projected directly into v^T[m,c] layout by using
    x_kv as the stationary operand. Softmax denominators come from a
    ones-column matmul over E; normalization is folded in after av via a
    broadcast matmul of the reciprocal row. The FFN for chunk j is emitted
    during chunk j+1's attention so the PE never waits on the softmax
    reciprocal chain.

Compute dtype: float32r (TF32-like, ~1.5e-4 matmul error, 1 cycle/row at
free-dim >= 256 — same speed as bf16). PSUM accumulation is fp32.
"""
import sys

sys.path.insert(0, "/opt/trn_rl_repo")

import numpy as np
from concourse import bass, bacc, mybir, tile
from concourse.bass_utils import run_bass_kernel_spmd

F32 = mybir.dt.float32
CDT = mybir.dt.float32r  # compute dtype for PE operands

B, C, H, W = 4, 256, 64, 64
N = H * W              # 4096 keys per batch
NL = N // 2            # 2048 query rows per core
CT = C // 128          # 2 channel tiles
MT = N // 128          # 32 key tiles
NCH = 512              # query-column chunk
NJ = NL // NCH         # 4 chunks
WPK = 5 * C + 4        # packed weight columns: 5 weights + 4 bias scalars
AF = mybir.ActivationFunctionType


def _build():
    nc = bacc.Bacc(None, target_bir_lowering=False, debug=False)

    xq_d = nc.declare_dram_parameter("xq", [128, CT, NL], F32, isOutput=False)
    xkv_d = nc.declare_dram_parameter("xkv", [128, CT, N], F32, isOutput=False)
    wp_d = nc.declare_dram_parameter("wpack", [128, CT, WPK], F32, isOutput=False)
    bv_d = nc.declare_dram_parameter("bvb", [128, C], F32, isOutput=False)
    out_d = nc.declare_dram_parameter("out", [128, CT, NL], F32, isOutput=True)

    with tile.TileContext(nc) as tc:
        with (
            tc.tile_pool(name="const", bufs=1) as cpool,
            tc.tile_pool(name="stage", bufs=3) as spool,
            tc.tile_pool(name="big", bufs=1) as bpool,
            tc.tile_pool(name="work", bufs=2) as wpool,
            tc.tile_pool(name="et", bufs=3) as epool,
            tc.tile_pool(name="psum", bufs=1, space="PSUM") as pp,
            tc.tile_pool(name="psum2", bufs=2, space="PSUM") as pp2,
        ):
            # ---- weights + biases: one packed DMA, cast to CDT ----
            wstage = spool.tile([128, CT, WPK], F32, tag="wstage", name="wstage",
                                bufs=1)
            nc.sync.dma_start(wstage[:], wp_d[:])
            w_r = cpool.tile([128, CT, 5 * C], CDT, tag="w_r")
            nc.vector.tensor_copy(w_r[:], wstage[:, :, 0:5 * C])
            bias_s = cpool.tile([128, CT, 4], F32, tag="bias_s")
            nc.vector.tensor_copy(bias_s[:], wstage[:, :, 5 * C:WPK])
            bv_s = cpool.tile([128, C], F32, tag="bv_s")
            nc.sync.dma_start(bv_s[:], bv_d[:])

            def wsl(idx, ct, osl):  # weight idx, channel tile, out-col slice
                return w_r[:, ct, idx * C + osl.start: idx * C + osl.stop]

            WQ, WK, WV, W1, W2 = range(5)
            BQ, BK, B1, B2 = range(4)

            ones_f = cpool.tile([128, 1], F32, tag="ones_f")
            nc.vector.memset(ones_f[:], 1.0)
            ones_r = cpool.tile([128, 1], CDT, tag="ones_r")
            nc.vector.tensor_copy(ones_r[:], ones_f[:])
            onesrow_f = cpool.tile([1, 128], F32, tag="onesrow_f")
            nc.vector.memset(onesrow_f[:], 1.0)
            onesrow = cpool.tile([1, 128], CDT, tag="onesrow")
            nc.vector.tensor_copy(onesrow[:], onesrow_f[:])

            # ---- inputs: chunked DMA + cast, projections right behind ----
            xkv_r = bpool.tile([128, CT, N], CDT, tag="xkv_r")
            xq_r = bpool.tile([128, CT, NL], CDT, tag="xq_r")
            k_r = bpool.tile([128, CT, N], CDT, tag="k_r")
            q_r = bpool.tile([128, CT, NL], CDT, tag="q_r")
            vt_r = bpool.tile([128, MT, C], CDT, tag="vt_r")

            def kproj(j):  # 512-col chunk j of k
                sl = slice(j * NCH, (j + 1) * NCH)
                for ct in range(CT):
                    ps = pp2.tile([128, NCH], F32, tag="st", name=f"pk{j}_{ct}", bufs=3)
                    for ci in range(CT):
                        nc.tensor.matmul(ps[:], wsl(WK, ci, slice(ct * 128, ct * 128 + 128)),
                                         xkv_r[:, ci, sl], start=(ci == 0), stop=(ci == CT - 1))
                    nc.vector.tensor_scalar_add(k_r[:, ct, sl], ps[:], bias_s[:, ct, BK:BK + 1])

            def vtproj(mi):
                ps = pp2.tile([128, C], F32, tag="st", name=f"pv{mi}", bufs=3)
                for ci in range(CT):
                    nc.tensor.matmul(ps[:], xkv_r[:, ci, mi * 128:(mi + 1) * 128],
                                     w_r[:, ci, WV * C:WV * C + C], start=(ci == 0), stop=(ci == CT - 1))
                nc.vector.tensor_add(vt_r[:, mi, :], ps[:], bv_s[:])

            def qproj(j):
                sl = slice(j * NCH, (j + 1) * NCH)
                for ct in range(CT):
                    ps = pp2.tile([128, NCH], F32, tag="st", name=f"pq{j}_{ct}", bufs=3)
                    for ci in range(CT):
                        nc.tensor.matmul(ps[:], wsl(WQ, ci, slice(ct * 128, ct * 128 + 128)),
                                         xq_r[:, ci, sl], start=(ci == 0), stop=(ci == CT - 1))
                    nc.vector.tensor_scalar_add(q_r[:, ct, sl], ps[:], bias_s[:, ct, BQ:BQ + 1])

            # interleave: kv chunk c -> K/VT proj for it; q chunk -> Q proj
            for c in range(4):
                st = spool.tile([128, CT, 1024], F32, tag="xstage", name=f"kvst{c}")
                nc.sync.dma_start(st[:], xkv_d[:, :, c * 1024:(c + 1) * 1024])
                nc.vector.tensor_copy(xkv_r[:, :, c * 1024:(c + 1) * 1024], st[:])
                if c < 2:
                    stq = spool.tile([128, CT, 1024], F32, tag="xstage", name=f"qst{c}")
                    nc.sync.dma_start(stq[:], xq_d[:, :, c * 1024:(c + 1) * 1024])
                    nc.vector.tensor_copy(xq_r[:, :, c * 1024:(c + 1) * 1024], stq[:])
                kproj(2 * c)
                kproj(2 * c + 1)
                for mi in range(8 * c, 8 * c + 8):
                    vtproj(mi)
                if c < 2:
                    qproj(2 * c)
                    qproj(2 * c + 1)

            # ---- attention (chunk j) with the previous chunk's FFN emitted in
            # staged pieces during this chunk's m-loop, each with ~2 key-tiles
            # of slack so the in-order PE stream never waits on DVE/ACT ----
            ffn_state = {}

            def ffn_stages(j):
                """Yield (mi_trigger, emit_fn) pieces for chunk j's FFN."""
                sl = slice(j * NCH, (j + 1) * NCH)
                st_ = {}

                def s_recip():
                    av0, av1, smp = ffn_state.pop(j)
                    st_["av"] = (av0, av1)
                    r = wpool.tile([1, NCH], CDT, tag="recip", name=f"recip{j}")
                    with nc.allow_low_precision(reason="f32r reciprocal is ~2^-13 accurate"):
                        nc.vector.reciprocal(r[:], smp[:])
                    st_["recip"] = r

                def s_rbp():
                    rbp = pp2.tile([128, NCH], F32, tag="ffn", name=f"rbp{j}", bufs=1)
                    nc.tensor.matmul(rbp[:], onesrow[:], st_["recip"][:],
                                     start=True, stop=True)
                    st_["rbp"] = rbp

                def s_avn():
                    rb = wpool.tile([128, NCH], F32, tag="rb", name=f"rb{j}", bufs=1)
                    nc.vector.tensor_copy(rb[:], st_["rbp"][:])
                    avn = wpool.tile([128, CT, NCH], CDT, tag="avn", name=f"avn{j}",
                                     bufs=1)
                    av0, av1 = st_["av"]
                    nc.vector.tensor_mul(avn[:, 0, :], av0[:], rb[:])
                    nc.vector.tensor_mul(avn[:, 1, :], av1[:], rb[:])
                    st_["avn"] = avn
                    st_["hid"] = wpool.tile([128, CT, NCH], CDT, tag="hid",
                                            name=f"hid{j}", bufs=1)
                    st_["outp"] = wpool.tile([128, CT, NCH], F32, tag="outp",
                                             name=f"outp{j}", bufs=1)

                def s_hid(ot):
                    def go():
                        hp = pp2.tile([128, NCH], F32, tag="ffn",
                                      name=f"hp{j}_{ot}", bufs=1)
                        for ci in range(CT):
                            nc.tensor.matmul(
                                hp[:], wsl(W1, ci, slice(ot * 128, ot * 128 + 128)),
                                st_["avn"][:, ci, :], start=(ci == 0), stop=(ci == CT - 1))
                        nc.scalar.activation(st_["hid"][:, ot, :], hp[:], AF.Relu,
                                             bias=bias_s[:, ot, B1:B1 + 1])
                    return go

                def s_out(ot):
                    def go():
                        op = pp2.tile([128, NCH], F32, tag="ffn",
                                      name=f"op{j}_{ot}", bufs=1)
                        for ci in range(CT):
                            nc.tensor.matmul(
                                op[:], wsl(W2, ci, slice(ot * 128, ot * 128 + 128)),
                                st_["hid"][:, ci, :], start=(ci == 0), stop=(ci == CT - 1))
                        nc.vector.tensor_scalar_add(st_["outp"][:, ot, :], op[:],
                                                    bias_s[:, ot, B2:B2 + 1])
                    return go

                def s_dma():
                    nc.sync.dma_start(out_d[:, :, sl], st_["outp"][:])

                return [(2, s_recip), (6, s_rbp), (8, s_avn),
                        (11, s_hid(0)), (13, s_hid(1)),
                        (15, s_out(0)), (17, s_out(1)), (19, s_dma)]

            for j in range(NJ):
                sl = slice(j * NCH, (j + 1) * NCH)
                av0 = pp.tile([128, NCH], F32, tag="av0", name=f"av0_{j}")
                av1 = pp.tile([128, NCH], F32, tag="av1", name=f"av1_{j}")
                smp = pp.tile([1, NCH], F32, tag="sum", name=f"smp{j}", bufs=2)
                pending = ffn_stages(j - 1) if j > 0 else []
                for mi in range(MT):
                    sp = pp2.tile([128, NCH], F32, tag="st", name=f"sp{j}_{mi}", bufs=3)
                    for ci in range(CT):
                        nc.tensor.matmul(sp[:], k_r[:, ci, mi * 128:(mi + 1) * 128],
                                         q_r[:, ci, sl], start=(ci == 0), stop=(ci == CT - 1))
                    et = epool.tile([128, NCH], CDT, tag="et", name=f"et{j}_{mi}")
                    nc.scalar.activation(et[:], sp[:], AF.Exp)
                    first, last = mi == 0, mi == MT - 1
                    nc.tensor.matmul(smp[:], ones_r[:], et[:], start=first, stop=last)
                    nc.tensor.matmul(av0[:], vt_r[:, mi, 0:128], et[:], start=first, stop=last)
                    nc.tensor.matmul(av1[:], vt_r[:, mi, 128:256], et[:], start=first, stop=last)
                    while pending and pending[0][0] == mi:
                        pending.pop(0)[1]()
                ffn_state[j] = (av0, av1, smp)
            for _, fn in ffn_stages(NJ - 1):
                fn()
    nc.compile()
    return nc


_NC_CACHE = None


def _get_nc():
    global _NC_CACHE
    if _NC_CACHE is None:
        _NC_CACHE = _build()
    return _NC_CACHE


def _fold(a):
    """[C, X] -> [128, CT, X] with channel tile as middle dim, contiguous."""
    x = np.ascontiguousarray(np.asarray(a, dtype=np.float32))
    return np.ascontiguousarray(x.reshape(CT, 128, -1).transpose(1, 0, 2))


def _make_in_maps(inputs):
    query_input = np.asarray(inputs["query_input"], np.float32).reshape(B, C, N)
    key_value_input = np.asarray(inputs["key_value_input"], np.float32).reshape(B, C, N)
    packs = [np.asarray(inputs[w], np.float32).T for w in ("Wq", "Wk", "Wv", "W1", "W2")]
    packs += [np.asarray(inputs[b], np.float32).reshape(C, 1)
              for b in ("bq", "bk", "b1", "b2")]
    wpack = _fold(np.concatenate(packs, axis=1))  # [128, CT, WPK]
    base = {
        "wpack": wpack,
        "bvb": np.ascontiguousarray(
            np.broadcast_to(np.asarray(inputs["bv"], np.float32)[None, :], (128, C))),
    }
    in_maps = []
    for core in range(8):
        b, h = divmod(core, 2)
        m = dict(base)
        m["xq"] = _fold(query_input[b][:, h * NL:(h + 1) * NL])
        m["xkv"] = _fold(key_value_input[b])
        in_maps.append(m)
    return in_maps


def kernel(query_input, key_value_input, Wq, bq, Wk, bk, Wv, bv, W1, b1, W2, b2):
    in_maps = _make_in_maps(dict(
        query_input=query_input, key_value_input=key_value_input,
        Wq=Wq, bq=bq, Wk=Wk, bk=bk, Wv=Wv, bv=bv, W1=W1, b1=b1, W2=W2, b2=b2))
    nc = _get_nc()
    res = run_bass_kernel_spmd(nc, in_maps, core_ids=list(range(8)))

    out = np.empty((B, C, N), dtype=np.float32)
    for core in range(8):
        b, h = divmod(core, 2)
        o = res.results[core]["out"]  # [128, CT, NL]
        out[b][:, h * NL:(h + 1) * NL] = o.transpose(1, 0, 2).reshape(C, NL)
    return out.reshape(B, C, H, W)



# revision 6
# speedup vs baseline: 1.1960x; 1.1960x over previous
"""Fused conv1x1-attention-FFN kernel for 8 trn2 NeuronCores.

Reference computation (per batch b of 4, N = 64*64 = 4096 pixels, C = 256):
    q = Wq @ x_q + bq ; k = Wk @ x_kv + bk ; v = Wv @ x_kv + bv      [C, N]
    attn = softmax_over_keys(q^T k)                                   [N, N]
    av = v @ attn^T                                                   [C, N]
    out = W2 @ relu(W1 @ av + b1) + b2                                [C, N]

Sharding: 8 cores = 4 batches x 2 query-row halves. Each core computes the
full K/V for its batch (cheap vs the [N,N] attention) and attends its 2048
query rows against all 4096 keys — no collectives.

Bias algebra (host-side): bk shifts every score of a query by the same
amount, which softmax cancels -> dropped. bv adds bv per channel to the
normalized attention output (weights sum to 1) -> folded into b1' = b1 +
W1 @ bv. Only bq, b1', b2 reach the device.

On-chip layout: scores are computed TRANSPOSED, S^T[m,n] = sum_c k[c,m]
q[c,n], so av[c,n] = sum_m v^T[m,c] E[m,n] needs no transpose; v is
projected directly into v^T[m,c] by using x_kv tiles as the stationary
operand. Softmax denominators are accumulated on the Vector engine
(acc += exp tile) and reduced over keys with ONE 1-column matmul per
chunk — the v1 kernel burned 128 full PE passes on this. Normalization
uses reciprocal_approx_fast (~18 bits) + a broadcast matmul of the
reciprocal row. The FFN for chunk j is interleaved into chunk j+1's
attention; the last chunk's FFN is split into two 256-col halves to
shorten the serial tail.

Everything DMAs straight into float32r tiles (f32r is bit-identical to
f32) — no staging casts. Compute dtype float32r: 1 PE cycle/row at
free-dim >= 256, ~2^-13 matmul rounding; PSUM accumulates fp32.
"""
import sys

sys.path.insert(0, "/opt/trn_rl_repo")

import numpy as np
from concourse import bass, bacc, mybir, tile
from concourse.bass_utils import run_bass_kernel_spmd

F32 = mybir.dt.float32
CDT = mybir.dt.float32r

B, C, H, W = 4, 256, 64, 64
N = H * W              # 4096 keys per batch
NL = N // 2            # 2048 query rows per core
CT = C // 128          # 2 channel tiles
MT = N // 128          # 32 key tiles
NCH = 512              # query-column chunk
NJ = NL // NCH         # 4 chunks
AF = mybir.ActivationFunctionType
BQ, B1, B2 = range(3)  # bias pack columns


def _build():
    nc = bacc.Bacc(None, target_bir_lowering=False, debug=False)

    xq_d = nc.declare_dram_parameter("xq", [128, CT, NL], CDT, isOutput=False)
    xkv_d = nc.declare_dram_parameter("xkv", [128, CT, N], CDT, isOutput=False)
    wp_d = nc.declare_dram_parameter("wpack", [128, CT, 5 * C], CDT, isOutput=False)
    bias_d = nc.declare_dram_parameter("biasp", [128, CT, 3], F32, isOutput=False)
    out_d = nc.declare_dram_parameter("out", [128, CT, NL], F32, isOutput=True)

    with tile.TileContext(nc) as tc:
        with (
            tc.tile_pool(name="const", bufs=1) as cpool,
            tc.tile_pool(name="big", bufs=1) as bpool,
            tc.tile_pool(name="work", bufs=2) as wpool,
            tc.tile_pool(name="et", bufs=3) as epool,
            tc.tile_pool(name="acc", bufs=2) as apool,
            tc.tile_pool(name="psA", bufs=1, space="PSUM") as pp,
            tc.tile_pool(name="psB", bufs=2, space="PSUM") as pp2,
        ):
            # ---- weights/biases straight to SBUF (f32r == f32 bits) ----
            w_r = cpool.tile([128, CT, 5 * C], CDT, tag="w_r")
            nc.sync.dma_start(w_r[:], wp_d[:])
            bias_s = cpool.tile([128, CT, 3], F32, tag="bias_s")
            nc.sync.dma_start(bias_s[:], bias_d[:])

            def wsl(idx, ct, osl):  # weight idx, contraction tile, out-col slice
                return w_r[:, ct, idx * C + osl.start: idx * C + osl.stop]

            WQ, WK, WV, W1, W2 = range(5)

            ones_f = cpool.tile([128, 1], F32, tag="ones_f")
            nc.vector.memset(ones_f[:], 1.0)
            ones_r = cpool.tile([128, 1], CDT, tag="ones_r")
            nc.vector.tensor_copy(ones_r[:], ones_f[:])
            onesrow_f = cpool.tile([1, 128], F32, tag="onesrow_f")
            nc.vector.memset(onesrow_f[:], 1.0)
            onesrow = cpool.tile([1, 128], CDT, tag="onesrow")
            nc.vector.tensor_copy(onesrow[:], onesrow_f[:])

            # ---- inputs: all DMAs issued up front, no staging casts ----
            xkv_r = bpool.tile([128, CT, N], CDT, tag="xkv_r")
            xq_r = bpool.tile([128, CT, NL], CDT, tag="xq_r")
            for c in range(4):
                nc.sync.dma_start(xkv_r[:, :, c * 1024:(c + 1) * 1024],
                                  xkv_d[:, :, c * 1024:(c + 1) * 1024])
                if c < 2:
                    nc.sync.dma_start(xq_r[:, :, c * 1024:(c + 1) * 1024],
                                      xq_d[:, :, c * 1024:(c + 1) * 1024])

            k_r = bpool.tile([128, CT, N], CDT, tag="k_r")
            q_r = bpool.tile([128, CT, NL], CDT, tag="q_r")
            vt_r = bpool.tile([128, MT, C], CDT, tag="vt_r")

            def kproj(j):  # 512-col chunk j of k (no bias — softmax-invariant)
                sl = slice(j * NCH, (j + 1) * NCH)
                for ct in range(CT):
                    ps = pp2.tile([128, NCH], F32, tag="st", name=f"pk{j}_{ct}", bufs=3)
                    for ci in range(CT):
                        nc.tensor.matmul(ps[:], wsl(WK, ci, slice(ct * 128, ct * 128 + 128)),
                                         xkv_r[:, ci, sl], start=(ci == 0), stop=(ci == CT - 1))
                    nc.vector.tensor_copy(k_r[:, ct, sl], ps[:])

            def vtproj(mi):  # v^T tile: [key, channel] layout, no bias
                ps = pp2.tile([128, C], F32, tag="st", name=f"pv{mi}", bufs=3)
                for ci in range(CT):
                    nc.tensor.matmul(ps[:], xkv_r[:, ci, mi * 128:(mi + 1) * 128],
                                     w_r[:, ci, WV * C:WV * C + C], start=(ci == 0), stop=(ci == CT - 1))
                nc.vector.tensor_copy(vt_r[:, mi, :], ps[:])

            def qproj(j):  # bias via ACT copy (per-partition bias)
                sl = slice(j * NCH, (j + 1) * NCH)
                for ct in range(CT):
                    ps = pp2.tile([128, NCH], F32, tag="st", name=f"pq{j}_{ct}", bufs=3)
                    for ci in range(CT):
                        nc.tensor.matmul(ps[:], wsl(WQ, ci, slice(ct * 128, ct * 128 + 128)),
                                         xq_r[:, ci, sl], start=(ci == 0), stop=(ci == CT - 1))
                    nc.scalar.activation(q_r[:, ct, sl], ps[:], AF.Identity,
                                         bias=bias_s[:, ct, BQ:BQ + 1])

            # ---- attention tile (j, mi): scores, exp, av accumulate, sum ----
            av_tiles = {}
            acc_tiles = {}
            smp_tiles = {}
            recip_tiles = {}

            def att_tile(j, mi):
                sl = slice(j * NCH, (j + 1) * NCH)
                if mi == 0:
                    av_tiles[j] = (
                        pp.tile([128, NCH], F32, tag="av0", name=f"av0_{j}", bufs=2),
                        pp.tile([128, NCH], F32, tag="av1", name=f"av1_{j}", bufs=2),
                    )
                    acc_tiles[j] = apool.tile([128, NCH], CDT, tag="acc",
                                              name=f"acc{j}")
                av0, av1 = av_tiles[j]
                sp = pp2.tile([128, NCH], F32, tag="st", name=f"sp{j}_{mi}", bufs=3)
                for ci in range(CT):
                    nc.tensor.matmul(sp[:], k_r[:, ci, mi * 128:(mi + 1) * 128],
                                     q_r[:, ci, sl], start=(ci == 0), stop=(ci == CT - 1))
                et = epool.tile([128, NCH], CDT, tag="et", name=f"et{j}_{mi}")
                nc.scalar.activation(et[:], sp[:], AF.Exp)
                first, last = mi == 0, mi == MT - 1
                nc.tensor.matmul(av0[:], vt_r[:, mi, 0:128], et[:], start=first, stop=last)
                nc.tensor.matmul(av1[:], vt_r[:, mi, 128:256], et[:], start=first, stop=last)
                acc = acc_tiles[j]
                if first:
                    nc.vector.tensor_copy(acc[:], et[:])
                else:
                    nc.vector.tensor_add(acc[:], acc[:], et[:])

            def denom(j):  # one 1-row matmul reduces acc over the key axis
                smp = pp2.tile([1, NCH], F32, tag="ffn", name=f"smp{j}", bufs=1)
                nc.tensor.matmul(smp[:], ones_r[:], acc_tiles.pop(j)[:],
                                 start=True, stop=True)
                smp_tiles[j] = smp

            # ---- FFN for chunk j, emitted piecewise during chunk j+1 ----
            def ffn_stages(j, half=None):
                """(trigger_mi, emit_fn) pieces; half splits the 512 cols."""
                st_ = {}
                hsl = slice(0, NCH) if half is None else \
                    slice(half * (NCH // 2), (half + 1) * (NCH // 2))
                osl = slice(j * NCH + hsl.start, j * NCH + hsl.stop)
                hn = hsl.stop - hsl.start

                def s_recip():
                    smp = smp_tiles.pop(j)
                    rt = wpool.tile([1, NCH], F32, tag="recip_f", name=f"recipf{j}")
                    with nc.allow_low_precision(reason="softmax denom needs ~8 bits"):
                        nc.vector.reciprocal_approx_fast(rt[:], smp[:])
                    r = wpool.tile([1, NCH], CDT, tag="recip", name=f"recip{j}")
                    nc.vector.tensor_copy(r[:], rt[:])
                    recip_tiles[j] = r

                def s_rbp():
                    rbp = pp2.tile([128, hn], F32, tag="ffn", name=f"rbp{j}_{half}",
                                   bufs=1)
                    nc.tensor.matmul(rbp[:], onesrow[:], recip_tiles[j][:, hsl],
                                     start=True, stop=True)
                    st_["rbp"] = rbp

                def s_avn():
                    rb = wpool.tile([128, hn], F32, tag="rb", name=f"rb{j}_{half}",
                                    bufs=2)
                    nc.vector.tensor_copy(rb[:], st_["rbp"][:])
                    avn = wpool.tile([128, CT, hn], CDT, tag="avn",
                                     name=f"avn{j}_{half}", bufs=2)
                    av0, av1 = av_tiles[j] if half in (None, 0) else av_tiles.pop(j)
                    nc.vector.tensor_mul(avn[:, 0, :], av0[:, hsl], rb[:])
                    nc.vector.tensor_mul(avn[:, 1, :], av1[:, hsl], rb[:])
                    if half is None:
                        av_tiles.pop(j)
                    st_["avn"] = avn
                    st_["hid"] = wpool.tile([128, CT, hn], CDT, tag="hid",
                                            name=f"hid{j}_{half}", bufs=2)
                    st_["outp"] = wpool.tile([128, CT, hn], F32, tag="outp",
                                             name=f"outp{j}_{half}", bufs=2)

                def s_hid(ot):
                    def go():
                        hp = pp2.tile([128, hn], F32, tag="ffn",
                                      name=f"hp{j}_{half}_{ot}", bufs=1)
                        for ci in range(CT):
                            nc.tensor.matmul(
                                hp[:], wsl(W1, ci, slice(ot * 128, ot * 128 + 128)),
                                st_["avn"][:, ci, :], start=(ci == 0), stop=(ci == CT - 1))
                        nc.scalar.activation(st_["hid"][:, ot, :], hp[:], AF.Relu,
                                             bias=bias_s[:, ot, B1:B1 + 1])
                    return go

                def s_out(ot):
                    def go():
                        op = pp2.tile([128, hn], F32, tag="ffn",
                                      name=f"op{j}_{half}_{ot}", bufs=1)
                        for ci in range(CT):
                            nc.tensor.matmul(
                                op[:], wsl(W2, ci, slice(ot * 128, ot * 128 + 128)),
                                st_["hid"][:, ci, :], start=(ci == 0), stop=(ci == CT - 1))
                        nc.vector.tensor_scalar_add(st_["outp"][:, ot, :], op[:],
                                                    bias_s[:, ot, B2:B2 + 1])
                    return go

                def s_dma():
                    nc.sync.dma_start(out_d[:, :, osl], st_["outp"][:])

                if half == 1:  # tail second half: recip/rbp already done
                    return [(0, s_rbp), (1, s_avn), (2, s_hid(0)), (3, s_hid(1)),
                            (4, s_out(0)), (5, s_out(1)), (6, s_dma)]
                return [(0, s_recip), (2, s_rbp), (4, s_avn),
                        (7, s_hid(0)), (9, s_hid(1)),
                        (11, s_out(0)), (13, s_out(1)), (15, s_dma)]

            # ---- schedule ----
            # streaming prologue: project each kv chunk as it lands and run
            # chunk-0 attention right behind it so the PE never drains.
            for c in range(4):
                kproj(2 * c)
                kproj(2 * c + 1)
                for mi in range(8 * c, 8 * c + 8):
                    vtproj(mi)
                if c == 0:
                    qproj(0)
                    qproj(1)
                if c == 1:
                    qproj(2)
                    qproj(3)
                for mi in range(8 * c, 8 * c + 8):
                    att_tile(0, mi)
            denom(0)

            for j in range(1, NJ):
                pending = ffn_stages(j - 1)
                for mi in range(MT):
                    att_tile(j, mi)
                    while pending and pending[0][0] == mi:
                        pending.pop(0)[1]()
                denom(j)

            # tail: last chunk's FFN in two 256-col halves to pipeline the
            # serial recip->rbp->avn->hid->out chain against itself.
            for _, fn in ffn_stages(NJ - 1, half=0):
                fn()
            for _, fn in ffn_stages(NJ - 1, half=1):
                fn()
    nc.compile()
    return nc


_NC_CACHE = None


def _get_nc():
    global _NC_CACHE
    if _NC_CACHE is None:
        _NC_CACHE = _build()
    return _NC_CACHE


def _fold(a):
    """[C, X] -> [128, CT, X] with channel tile as middle dim, contiguous."""
    x = np.ascontiguousarray(np.asarray(a, dtype=np.float32))
    return np.ascontiguousarray(x.reshape(CT, 128, -1).transpose(1, 0, 2))


def _make_in_maps(inputs):
    query_input = np.asarray(inputs["query_input"], np.float32).reshape(B, C, N)
    key_value_input = np.asarray(inputs["key_value_input"], np.float32).reshape(B, C, N)
    wpack = _fold(np.concatenate(
        [np.asarray(inputs[w], np.float32).T for w in ("Wq", "Wk", "Wv", "W1", "W2")],
        axis=1))  # [128, CT, 5C]
    W1_ = np.asarray(inputs["W1"], np.float32)
    b1p = np.asarray(inputs["b1"], np.float32) + W1_ @ np.asarray(inputs["bv"], np.float32)
    biasp = _fold(np.stack(
        [np.asarray(inputs["bq"], np.float32), b1p,
         np.asarray(inputs["b2"], np.float32)], axis=1))  # [128, CT, 3]
    base = {"wpack": wpack, "biasp": biasp}
    in_maps = []
    for core in range(8):
        b, h = divmod(core, 2)
        m = dict(base)
        m["xq"] = _fold(query_input[b][:, h * NL:(h + 1) * NL])
        m["xkv"] = _fold(key_value_input[b])
        in_maps.append(m)
    return in_maps


def kernel(query_input, key_value_input, Wq, bq, Wk, bk, Wv, bv, W1, b1, W2, b2):
    in_maps = _make_in_maps(dict(
        query_input=query_input, key_value_input=key_value_input,
        Wq=Wq, bq=bq, Wk=Wk, bk=bk, Wv=Wv, bv=bv, W1=W1, b1=b1, W2=W2, b2=b2))
    nc = _get_nc()
    res = run_bass_kernel_spmd(nc, in_maps, core_ids=list(range(8)))

    out = np.empty((B, C, N), dtype=np.float32)
    for core in range(8):
        b, h = divmod(core, 2)
        o = res.results[core]["out"]  # [128, CT, NL]
        out[b][:, h * NL:(h + 1) * NL] = o.transpose(1, 0, 2).reshape(C, NL)
    return out.reshape(B, C, H, W)
